# revision 56
# baseline (speedup 1.0000x reference)
"""Criss-cross attention (2-stream) Trainium2 kernel.

Data-parallel over batch B=8 across 8 NeuronCores; one image pair per core.

Per-core algorithm (all matmuls bf16, fp32 PSUM accumulation):
  - q/k projections for both streams in one pass (bias host-corrected)
  - transposed logits E^T per column (diag mask PRELOADED into PSUM via a
    matmul so the logit matmuls accumulate on top of it) / per row,
    joint softmax without max-subtraction (logits are O(30); exp safe in f32)
  - Z-trick: Z[c',p] = sum_g x[c',g] * Phat[p,g] using host-supplied
    spatially-transposed x copies, then one dense (gamma*wv) @ Z projection.
    v-bias folds out exactly because joint softmax weights sum to 1:
      attn = wv@Z + bv;  out = gamma*attn + x = (gamma*wv)@Z + (x + gamma*bv)
    with x~ = x + gamma*bv supplied by host (and bq' = bq - wq@(gamma*bv),
    bk' = bk - wk@(gamma*bv) correcting the q/k projections).

Perf notes (550us -> ~360us on HW):
  - Cross-attend pipelining: attend 1's col/row logit blocks are emitted
    interleaved with attend 0's zcol/zrow slabs so their exps chase the
    Z reads chunk-by-chunk; proj groups interleave with zrow slabs; the
    mc=1 residual rides PE (identity preload) + ACT copy; stats run on
    DVE reciprocal_approx_fast (~18 correct bits).  Engine queues stay
    fed instead of serializing phase-by-phase.
  - Strided-AP elimination (the dominant unmodeled HW cost): zcol/row
    PSUM outputs are written spatially INTERLEAVED by the PE (strided
    PSUM out APs are ~free) so the ACT/DVE egress reads dense PSUM and
    writes >=4-element runs instead of 2-byte scatters.  This alone was
    worth ~25%.
  - xtc/xtr x-copies ship as fp8 e3m4 (half the DMA bytes; x~N(0,1)
    fits e3m4's range with ~1.5% element error that averages out).
  - exp/ln/identity share one activation-table set (_patch_act_tables).
  - diag mask applied by PE (PSUM preload); FWL-padded row stationaries.
  - eps pool runs 4x single-bank PSUM slots for deeper producer/consumer
    handoff pipelining.
  - attend 0's q/k extract + row blocks chunked into the (DMA-bound)
    projection loop.
  - test.py times via median of same-round pairwise For_i-loop slopes
    (robust to the tunnel's bimodal per-call overhead).
"""

import sys

sys.path.insert(0, "/opt/trn_rl_repo")

import numpy as np
import ml_dtypes


def _patch_act_tables():
    """Make Exp/Ln resolve to the combined natural_log_exp_and_others
    activation-table set so alternating ln/exp does not reload tables.

    bass picks the first set whose membership contains the function; by
    default Exp -> exp_and_others and Ln -> natural_log, which forces a
    ~2.7us table reload on every ln<->exp switch.  Dropping Exp/Ln from
    the other sets' advertised membership makes both resolve to the one
    real hardware set that contains both functions."""
    import functools
    import concourse.hw_specs as hw_specs
    from concourse import mybir

    if getattr(hw_specs.get_activation_tables, "_cc_patched", False):
        return
    orig = hw_specs.get_activation_tables.__wrapped__
    EXP = mybir.ActivationFunctionType.Exp
    LN = mybir.ActivationFunctionType.Ln
    IDENT = mybir.ActivationFunctionType.Identity

    def patched(module_arch):
        out = {}
        for name, fns in orig(module_arch).items():
            fns = set(fns)
            if name != "natural_log_exp_and_others":
                fns.discard(EXP)
                fns.discard(LN)
                fns.discard(IDENT)
            out[name] = fns
        return out

    wrapper = functools.cache(patched)
    wrapper._cc_patched = True
    hw_specs.get_activation_tables = wrapper


_patch_act_tables()

BF = ml_dtypes.bfloat16
F8 = ml_dtypes.float8_e3m4
B, C, H, W = 8, 256, 96, 96
CQ = 32
S = H * W  # 9216
NEG = -1.0e30
SLW = 24  # spatial slices per xT slab

_CACHE = {}
import os as _os
_qe = _os.environ.get("QKFOLD", "0")
DENSE_ZROW = _os.environ.get("DENSE_ZROW", "0") == "1"  # timing ablation only
DENSE_COL = _os.environ.get("DENSE_COL", "0") == "1"    # timing ablation only
QKFOLD = _qe != "0"          # extract4 + qs4/ks4 maintained
QKFOLD_MM = _qe == "2"       # col matmuls consume the folded layout



def build_nc(reps=1, stop_after=99):
    import concourse.tile as tile
    from concourse import bacc, mybir

    f32 = mybir.dt.float32
    bf16 = mybir.dt.bfloat16

    nc = bacc.Bacc("TRN2", target_bir_lowering=False, debug=False, num_devices=8)

    din = {}

    def dparam(name, shape, dt=bf16):
        din[name] = nc.dram_tensor(name, shape, dt, kind="ExternalInput").ap()

    dparam("xa0", [C, S])          # bf16(x0 + g*bv0), channel-major
    dparam("xa1", [C, S])
    f8 = mybir.dt.float8e3
    dparam("xtc0", [H, W * C], f8)  # xtc[h, w*256+c] = x0[c,h,w]  (raw x)
    dparam("xtr0", [W, H * C], f8)  # xtr[w, h*256+c] = x0[c,h,w]
    dparam("xtc1", [H, W * C], f8)
    dparam("xtr1", [W, H * C], f8)
    dparam("wqk", [C, 128])        # cols: wq0T|wk0T|wq1T|wk1T
    dparam("wva", [C, 512])        # (gamma*wv0).T | (gamma*wv1).T
    dparam("qkb", [128, 1], f32)   # bq0'|bk0'|bq1'|bk1'
    dparam("maskw", [H, 128])      # -1e30 diag (cols 96:128 zero)
    dparam("idm", [H, 384])        # identity tiled 4x along cols
    dparam("ident", [128, 128])    # identity (psum residual preload)
    out = nc.dram_tensor("out", [2, C, S], bf16, kind="ExternalOutput").ap()

    with tile.TileContext(nc) as tc:
        if reps == 1:
            _emit(tc, nc, din, out, mybir, stop_after)
        else:
            # the body is far over 256 instructions per engine (≈12 IRAM
            # blocks on PE), so the back-edge target always I$-misses
            # (~3-4us/iter); branch-prefetch hints cut that to ~300ns.
            from concourse.engine_type import EngineType
            with tc.For_i(0, reps, 1,
                          hint_engines=(EngineType.PE, EngineType.Activation,
                                        EngineType.DVE)):
                _emit(tc, nc, din, out, mybir, stop_after)

    nc.compile()
    return nc


def _emit(tc, nc, din, out, mybir, stop_after=99):
    from contextlib import ExitStack

    f32 = mybir.dt.float32
    bf16 = mybir.dt.bfloat16
    f8 = mybir.dt.float8e3
    EXP = mybir.ActivationFunctionType.Exp
    LN = mybir.ActivationFunctionType.Ln
    CPY = mybir.ActivationFunctionType.Copy
    IDENT = mybir.ActivationFunctionType.Identity
    ADD = mybir.AluOpType.add
    MUL = mybir.AluOpType.mult

    SP = S + 128  # padded q/k width (row-logit FWL reads h*96..h*96+128)

    ctx = ExitStack()
    with ctx:
        const = ctx.enter_context(tc.tile_pool(name="const", bufs=1))
        persist = ctx.enter_context(tc.tile_pool(name="persist", bufs=1))
        # PSUM: eps 4x1bank + lps 2x2banks = all 8 banks.  eps runs 4
        # single-bank slots so producer->consumer handoffs pipeline deeper
        # (cross-engine handoff latency is the dominant unmodeled HW cost).
        eps = ctx.enter_context(tc.tile_pool(name="eps", bufs=4, space="PSUM"))
        lps = ctx.enter_context(tc.tile_pool(name="lps", bufs=2, space="PSUM"))

        # ---------------- constants (ACT-triggered HWDGE queue) ----------
        wqk_t = []
        for kc in range(2):
            t = const.tile([128, 128], bf16, tag=f"wqk{kc}", name=f"wqk{kc}")
            nc.scalar.dma_start(t[:], din["wqk"][kc * 128:(kc + 1) * 128, :])
            wqk_t.append(t)
        # wv weights for both streams, one [128, 512] tile per kc half
        wva_t = []
        for kc in range(2):
            t = const.tile([128, 512], bf16, tag=f"wva{kc}", name=f"wva{kc}")
            nc.scalar.dma_start(t[:], din["wva"][kc * 128:(kc + 1) * 128, :])
            wva_t.append(t)
        wv_t = [
            [
                [wva_t[kc][:, s * 256 + mc * 128:s * 256 + (mc + 1) * 128]
                 for mc in range(2)]
                for kc in range(2)
            ]
            for s in range(2)
        ]
        qkb_t = const.tile([128, 1], f32, tag="qkb")
        nc.scalar.dma_start(qkb_t[:], din["qkb"][:])
        maskw_t = const.tile([H, 128], bf16, tag="maskw")
        nc.scalar.dma_start(maskw_t[:], din["maskw"][:])
        idm_t = const.tile([H, 384], bf16, tag="idm")
        nc.scalar.dma_start(idm_t[:], din["idm"][:])
        ones_t = const.tile([H, 128], bf16, tag="ones")
        nc.vector.memset(ones_t[:], 1.0)
        ident_t = const.tile([128, 128], bf16, tag="ident")
        nc.scalar.dma_start(ident_t[:], din["ident"][:])

        # persistent state
        qk_t = persist.tile([128, S], bf16, tag="qk")
        qs = persist.tile([CQ, SP], bf16, tag="qs")
        ks = persist.tile([CQ, SP], bf16, tag="ks")
        ks4 = persist.tile([CQ, S], bf16, tag="ks4") if QKFOLD else None
        pcol = persist.tile([H, S], bf16, tag="pcol")
        prow = persist.tile([W, S], bf16, tag="prow")
        z = [persist.tile([128, S], bf16, tag=f"z{kc}", name=f"z{kc}")
             for kc in range(2)]

        # zero the FWL pad columns once up-front (extract DMAs never touch
        # them)
        nc.vector.memset(qs[:, S:SP], 0.0)
        nc.vector.memset(ks[:, S:SP], 0.0)

        # ---------------- q/k projections (xa streamed in 3K slabs) -----
        # qk_t rows: q0(0:32) k0(32:64) q1(64:96) k1(96:128)
        # attend 0's q/k extract is chunked into this loop, and attend 0's
        # row-logit blocks whose FWL-padded reads fit inside the extracted
        # prefix are emitted right behind each chunk, so PE/ACT start attend
        # 0 while xa is still streaming in.  (Called at emission time below;
        # row_block is late-bound.)
        ROWB = {0: tuple(range(0, 28, 4)), 1: tuple(range(28, 60, 4)),
                2: tuple(range(60, 96, 4))}

        def qk_proj():
            with tc.tile_pool(name="xslp", bufs=2) as xslp:
                for n3 in range(3):
                    xsl = [[None] * 2 for _ in range(2)]
                    for s in range(2):
                        for kc in range(2):
                            t = xslp.tile([128, 3072], bf16, tag=f"x{s}s{kc}",
                                          name=f"x{s}s{kc}")
                            xsl[s][kc] = t
                            nc.sync.dma_start(
                                t[:],
                                din[f"xa{s}"][kc * 128:(kc + 1) * 128,
                                              n3 * 3072:(n3 + 1) * 3072],
                            )
                    for j2 in range(6):
                        jsl = slice(j2 * 512, (j2 + 1) * 512)
                        p = eps.tile([128, 512], f32, tag="eps", name="p")
                        for kc in range(2):
                            nc.tensor.matmul(
                                p[0:64, :], wqk_t[kc][:, 0:64],
                                xsl[0][kc][:, jsl],
                                start=(kc == 0), stop=(kc == 1),
                            )
                        for kc in range(2):
                            nc.tensor.matmul(
                                p[64:128, :], wqk_t[kc][:, 64:128],
                                xsl[1][kc][:, jsl],
                                start=(kc == 0), stop=(kc == 1),
                                tile_position=(0, 64),
                                skip_group_check=True,
                            )
                        # PSUM egress + bias on DVE: ACT's queue holds the
                        # extract-gated row exps, so ACT egress here would
                        # starve the eps slots behind them (head-of-line).
                        nc.vector.tensor_scalar_add(
                            qk_t[:, n3 * 3072 + j2 * 512:
                                 n3 * 3072 + (j2 + 1) * 512], p[:], qkb_t[:]
                        )
                    n3sl = slice(n3 * 3072, (n3 + 1) * 3072)
                    nc.scalar.dma_start(qs[:, n3sl], qk_t[64:96, n3sl])
                    nc.scalar.dma_start(ks[:, n3sl], qk_t[32:64, n3sl])
                    # row blocks ride the (idle-during-qk) lps pool so their
                    # ets — which wait on the extract DMA — never hold the
                    # eps slots the projection's p tiles cycle through.
                    for h0 in ROWB[n3]:
                        row_block(0, h0, lps, "lps")

        # transient pools for the attend phases are entered after qk_proj()
        # runs (xslp's address range frees for them); the closures below
        # late-bind these names.
        slab = tsl = resl = obuf = None

        # ---------------- attends (phase closures, interleaved emission) --
        # pixel (h,w) lives at free index w*96+h in pcol/prow
        prow_hw = prow[:].rearrange("p (w h) -> p h w", h=H)
        prow_wh = prow[:].rearrange("p (w h) -> p w h", h=H)
        qs_wh = qs[:, 0:S].rearrange("p (h w) -> p w h", w=W)
        ks_wh = ks[:, 0:S].rearrange("p (h w) -> p w h", w=W)
        z_wh = [zz[:].rearrange("p (h w) -> p w h", w=W) for zz in z]
        z_hw = [zz[:].rearrange("p (h w) -> p h w", w=W) for zz in z]

        def extract(a):
            # PE requires matmul operands at equal base partitions: copy the
            # attend's q/k blocks to partition-0-based tiles (SBUF->SBUF DMA
            # on the otherwise-idle ACT-triggered queue).
            # (attend 0's extract is chunked into the q/k projection loop.)
            qr = 64 if a == 0 else 0    # query rows (q1 / q0)
            kr = 32 if a == 0 else 96   # key rows (k0 / k1)
            nc.scalar.dma_start(qs[:, 0:S], qk_t[qr:qr + 32, :])
            nc.scalar.dma_start(ks[:, 0:S], qk_t[kr:kr + 32, :])
            if QKFOLD:
                extract4(a)

        def extract4(a):
            # w%4-folded q/k copies for the col branch: free index
            # m*576 + (w//4)*96 + h (m = w%4).  Both col-matmul operands
            # become dense contiguous [32, 96] slices at base partition 0,
            # replacing the stride-96 reads of the w-major views; the
            # extract DMA moves contiguous 96-element runs.
            kr = 32 if a == 0 else 96
            for dst4, r in ((ks4, kr),):
                srcv = qk_t[r:r + 32, :].rearrange(
                    "p (w4 m h) -> p m w4 h", m=4, h=H
                )
                dstv = dst4[:, :].rearrange(
                    "p (m w4 h) -> p m w4 h", m=4, h=H
                )
                for m in range(4):
                    nc.scalar.dma_start(dstv[:, m], srcv[:, m])

        def stats_chunk(a, n2, dve_recip=False):
            sl = slice(n2 * 1024, (n2 + 1) * 1024)
            lt = lps.tile([128, 1024], f32, tag="lps", name="lt")
            for b in range(2):
                s512 = slice(n2 * 1024 + b * 512, n2 * 1024 + (b + 1) * 512)
                bank = lt[:, b * 512:(b + 1) * 512]
                nc.tensor.matmul(
                    bank, ones_t[:], pcol[:, s512],
                    start=True, stop=False, skip_group_check=True,
                )
                nc.tensor.matmul(
                    bank, ones_t[:], prow[:, s512],
                    start=False, stop=True, skip_group_check=True,
                )
            rs = tsl.tile([96, 1024], bf16, tag="rsl")
            if dve_recip:
                rr = tsl.tile([96, 1024], f32, tag="tln", name="rr")
                nc.vector.reciprocal_approx_fast(out=rr[:], in_=lt[0:96, :])
                nc.vector.tensor_copy(rs[:], rr[:])
            else:
                tl = tsl.tile([96, 1024], f32, tag="tln")
                nc.scalar.activation(tl[:], lt[0:96, :], LN)
                nc.scalar.activation(rs[:], tl[:], EXP, scale=-1.0)
            nc.vector.tensor_tensor(pcol[:, sl], pcol[:, sl], rs[:], MUL)
            nc.vector.tensor_tensor(prow[:, sl], prow[:, sl], rs[:], MUL)

        def row_block(a, h0, pool=None, tag="eps"):
            # 4 row-branch logit matmuls (FWL-padded stationary) + exp.
            # PE writes h-interleaved (stride-4 PSUM) so the exp reads dense
            # and writes 4-element runs into w-major prow instead of a
            # 2-byte scatter.
            et = (pool or eps).tile([128, 512], f32, tag=tag, name="et")
            etv = et[:, 0:384].rearrange("p (g j) -> p j g", j=4)
            for j in range(4):
                h = h0 + j
                nc.tensor.matmul(
                    etv[:, j, :],
                    ks[:, h * 96:h * 96 + 128],
                    qs[:, h * 96:(h + 1) * 96],
                    start=True, stop=True, skip_group_check=True,
                )
            src = et[0:96, 0:384].rearrange("p (g j) -> p g j", j=4)
            dst = prow_wh[:, :, h0:h0 + 4]
            nc.scalar.activation(dst, src, EXP)

        def col_block(a, w0, pool, tag):
            # diag-mask PSUM preload + 4 col-branch logit matmuls + exp.
            # With QKFOLD, w0 is a multiple of 4 and the 4 w's live at the 4
            # partition groups of qs4/ks4 with a shared (w//4) free slice,
            # so stationary and moving are dense and at the same base.
            w4 = w0 // 4
            fsl = slice(w4 * 96, (w4 + 1) * 96)
            et = pool.tile([128, 512], f32, tag=tag, name="et")
            nc.tensor.matmul(
                et[:, 0:384], maskw_t[:], idm_t[:],
                start=True, stop=False, skip_group_check=True,
            )
            for j in range(4):
                if QKFOLD_MM:
                    fo = j * 2304 + w4 * 96
                    nc.tensor.matmul(
                        et[0:96, j * 96:j * 96 + 96],
                        ks4[:, fo:fo + 96],
                        qs_wh[:, w0 + j, :],
                        start=False, stop=(j == 3), skip_group_check=True,
                    )
                else:
                    w = w0 + j
                    st = (ks[:, w * 96:w * 96 + 96] if DENSE_COL
                          else ks_wh[:, w, :])
                    mv = (qs[:, w * 96:w * 96 + 96] if DENSE_COL
                          else qs_wh[:, w, :])
                    nc.tensor.matmul(
                        et[0:96, j * 96:j * 96 + 96],
                        st, mv,
                        start=False, stop=(j == 3), skip_group_check=True,
                    )
            src = et[0:96, 0:384]
            nc.scalar.activation(
                pcol[:, w0 * 96:(w0 + 4) * 96], src, EXP
            )

        def zcol_slabs(a):
            # Z column branch: per w, Z[c', h]; scatter w-strided into z.
            # Generator yielding after each w-slab so attend 1's col logit
            # blocks can interleave (they chase this branch's pcol reads).
            xtc = din[f"xtc{a}"][:].rearrange("p (w c) -> p w c", c=C)
            for w0 in range(0, W, SLW):
                xs = slab.tile([H, SLW * 256], f8, tag="xslab", name="xs")
                nc.gpsimd.dma_start(xs[:], xtc[:, w0:w0 + SLW, :])
                for kc in range(2):
                    for g0 in range(0, SLW, 4):
                        zp = eps.tile([128, 512], f32, tag="eps", name="zp")
                        # PE writes w-interleaved (stride-4 PSUM) so the ACT
                        # egress reads dense and writes 4-element runs into
                        # z's h-major layout instead of a 2-byte scatter.
                        zpv = zp[:, 0:384].rearrange("p (g j) -> p j g", j=4)
                        for j in range(4):
                            wl = g0 + j
                            nc.tensor.matmul(
                                zpv[:, j, :],
                                xs[:, wl * 256 + kc * 128:wl * 256 + kc * 128 + 128],
                                pcol[:, (w0 + wl) * 96:(w0 + wl + 1) * 96],
                                start=True, stop=True, skip_group_check=True,
                            )
                        src = zp[:, 0:384].rearrange("p (g j) -> p g j", j=4)
                        dst = z_hw[kc][:, :, w0 + g0:w0 + g0 + 4]
                        nc.scalar.activation(dst, src, CPY)
                yield

        def zrow_slabs(a):
            # Z row branch: per h, Z[c', w]; accumulate into z.  Generator
            # yielding after each h-slab so proj emission can interleave.
            xtr = din[f"xtr{a}"][:].rearrange("p (h c) -> p h c", c=C)
            for h0 in range(0, H, SLW):
                xs = slab.tile([W, SLW * 256], f8, tag="xslab", name="xs")
                nc.sync.dma_start(xs[:], xtr[:, h0:h0 + SLW, :])
                for kc in range(2):
                    for g0 in range(0, SLW, 4):
                        zp = eps.tile([128, 512], f32, tag="eps", name="zp")
                        for j in range(4):
                            hl = g0 + j
                            mv = (prow[:, (h0 + hl) * 96:(h0 + hl + 1) * 96]
                                  if DENSE_ZROW else prow_hw[:, h0 + hl, :])
                            nc.tensor.matmul(
                                zp[:, j * 96:j * 96 + 96],
                                xs[:, hl * 256 + kc * 128:hl * 256 + kc * 128 + 128],
                                mv,
                                start=True, stop=True, skip_group_check=True,
                            )
                        zsl = z[kc][:, (h0 + g0) * 96:(h0 + g0 + 4) * 96]
                        src = zp[:, 0:384]
                        nc.vector.tensor_tensor(zsl, src, zsl, ADD)
                yield

        def proj_groups(a, act_split=False):
            # final projection + residual + store, as a generator yielding
            # after each (n3, mc) output group so callers can interleave other
            # phases' emissions between groups.  n3 is the outer loop so the
            # zrow interleave can release spatially-complete z ranges early.
            for n3 in range(3):
                for mc in range(2):
                    act_path = act_split and mc == 1
                    ob = obuf.tile([128, 3072], bf16, tag="ob")
                    rt = resl.tile([128, 3072], bf16, tag="res", name="rt")
                    nc.sync.dma_start(
                        rt[:],
                        din[f"xa{a}"][mc * 128:(mc + 1) * 128,
                                      n3 * 3072:(n3 + 1) * 3072],
                    )
                    for j2 in range(3):
                        n2 = n3 * 3 + j2
                        op = lps.tile([128, 1024], f32, tag="lps", name="op")
                        for b in range(2):
                            s512 = slice(n2 * 1024 + b * 512,
                                         n2 * 1024 + (b + 1) * 512)
                            opb = op[:, b * 512:(b + 1) * 512]
                            if act_path:
                                nc.tensor.matmul(
                                    opb, ident_t[:],
                                    rt[:, j2 * 1024 + b * 512:
                                       j2 * 1024 + (b + 1) * 512],
                                    start=True, stop=False,
                                    skip_group_check=True,
                                )
                            for kc in range(2):
                                nc.tensor.matmul(
                                    opb, wv_t[a][kc][mc], z[kc][:, s512],
                                    start=(not act_path and kc == 0),
                                    stop=(kc == 1),
                                )
                        jsl = slice(j2 * 1024, (j2 + 1) * 1024)
                        if act_path:
                            nc.scalar.activation(ob[:, jsl], op[:], CPY)
                        else:
                            nc.vector.tensor_tensor(
                                ob[:, jsl], op[:], rt[:, jsl], ADD
                            )
                    nc.gpsimd.dma_start(
                        out[a, mc * 128:(mc + 1) * 128,
                            n3 * 3072:(n3 + 1) * 3072],
                        ob[:],
                    )
                    yield

        def zrow_proj(a, next_rows=False):
            # zrow slabs interleaved with (optionally) the next attend's row
            # logit blocks — whose prow writes chase this branch's per-slab
            # reads — and with proj groups: proj group (n3, mc) needs z rows
            # h < (n3+1)*32 finalized, i.e. zrow slabs through
            # h0 = ceil((n3+1)*32 / SLW).  The residual path for mc=1 rides
            # PE (identity preload) + ACT copy so the DVE queue only carries
            # the zrow adds + mc=0 residuals.
            pg = proj_groups(a, act_split=True)
            zr = zrow_slabs(a)
            for si in range(4):          # slabs h0 = 0, 24, 48, 72
                next(zr)
                if next_rows:
                    for h0 in range(24 * si, 24 * si + 24, 4):
                        row_block(1 - a, h0)
                if si >= 1:
                    next(pg)             # (n3 = si-1, mc = 0)
                    next(pg)             # (n3 = si-1, mc = 1)
            for _ in pg:
                pass

        # Emission program.  The two attends are pipelined across engines:
        #  - attend 0's q/k extract and row-logit blocks are chunked into the
        #    projection loop above, so ACT starts exp'ing while xa streams in.
        #  - attend 0 col blocks + fused stats (recip on DVE, normalize on
        #    GpSimd) follow; then extract(1) (WAR on qs/ks clears when
        #    attend 0's logit matmuls retire).
        #  - zcol(0) slabs interleave with attend 1's col blocks: the col
        #    exps overwrite pcol w-block by w-block right behind zcol(0)'s
        #    w-ordered reads, and attend 1's logit matmuls fill PE while
        #    zcol(0)'s groups pace on ACT egress.  Attend 1's ets live in
        #    the lps pool so the two pipelines don't share PSUM slots.
        #  - zrow(0) slabs interleave with attend 1's row blocks (same
        #    chasing argument) and with proj(0) groups.
        #  - proj residuals for mc=1 ride PE (identity preload) + ACT copy;
        #    mc=0 stays on DVE.
        qk_proj()
        if stop_after <= 1:
            return
        slab = ctx.enter_context(tc.tile_pool(name="slab", bufs=3))
        tsl = ctx.enter_context(tc.tile_pool(name="tsl", bufs=2))
        resl = ctx.enter_context(tc.tile_pool(name="resl", bufs=2))
        obuf = ctx.enter_context(tc.tile_pool(name="obuf", bufs=2))
        # stats0 uses the ACT ln/exp path (its window is DVE-bound with ACT
        # slack); stats1 uses the DVE reciprocal (its window is the reverse).
        n2_done = 0
        for i, w0 in enumerate(range(0, W, 4)):
            col_block(0, w0, eps, "eps")
            cov = (w0 + 4) * 96
            while n2_done < 9 and (n2_done + 1) * 1024 <= cov:
                stats_chunk(0, n2_done, dve_recip=True)
                n2_done += 1
        while n2_done < 9:
            stats_chunk(0, n2_done, dve_recip=True)
            n2_done += 1
        if stop_after <= 2:
            return
        extract(1)
        zc = zcol_slabs(0)
        for si in range(4):
            next(zc)
            for w0 in range(24 * si, 24 * si + 24, 4):
                col_block(1, w0, lps, "lps")
        if stop_after <= 3:
            return
        zrow_proj(0, next_rows=True)
        if stop_after <= 4:
            return
        for n2 in range(9):
            stats_chunk(1, n2, dve_recip=True)
        zc = zcol_slabs(1)
        for si in range(4):
            next(zc)
        if stop_after <= 5:
            return
        zrow_proj(1)


def prep_inputs(inputs):
    """Host-side per-core input prep (numpy)."""
    g = float(np.asarray(inputs["gamma"]).reshape(-1)[0])
    maskw = np.zeros((H, 128), np.float32)
    np.fill_diagonal(maskw[:, 0:H], NEG)
    idm = np.tile(np.eye(H, dtype=np.float32), (1, 4))
    ident = np.eye(128, dtype=np.float32)
    wqk = np.concatenate(
        [inputs["wq0"].T, inputs["wk0"].T, inputs["wq1"].T, inputs["wk1"].T],
        axis=1,
    ).astype(BF)
    wv0 = (g * np.asarray(inputs["wv0"], np.float64)).T.astype(BF)
    wv1 = (g * np.asarray(inputs["wv1"], np.float64)).T.astype(BF)
    gb0 = g * np.asarray(inputs["bv0"], np.float64)
    gb1 = g * np.asarray(inputs["bv1"], np.float64)
    qkb = np.concatenate(
        [
            inputs["bq0"] - inputs["wq0"].astype(np.float64) @ gb0,
            inputs["bk0"] - inputs["wk0"].astype(np.float64) @ gb0,
            inputs["bq1"] - inputs["wq1"].astype(np.float64) @ gb1,
            inputs["bk1"] - inputs["wk1"].astype(np.float64) @ gb1,
        ]
    ).astype(np.float32)[:, None]
    maps = []
    for b in range(B):
        x0 = np.asarray(inputs["x0"][b], np.float32)
        x1 = np.asarray(inputs["x1"][b], np.float32)
        maps.append({
            "xa0": (x0 + np.float32(gb0[:, None, None])).reshape(C, S).astype(BF),
            "xa1": (x1 + np.float32(gb1[:, None, None])).reshape(C, S).astype(BF),
            "xtc0": np.ascontiguousarray(x0.transpose(1, 2, 0)).reshape(H, W * C).astype(F8),
            "xtr0": np.ascontiguousarray(x0.transpose(2, 1, 0)).reshape(W, H * C).astype(F8),
            "xtc1": np.ascontiguousarray(x1.transpose(1, 2, 0)).reshape(H, W * C).astype(F8),
            "xtr1": np.ascontiguousarray(x1.transpose(2, 1, 0)).reshape(W, H * C).astype(F8),
            "wqk": wqk, "wva": np.concatenate([wv0, wv1], axis=1), "qkb": qkb,
            "maskw": maskw.astype(BF), "idm": idm.astype(BF),
            "ident": ident.astype(BF),
        })
    return maps


def postprocess(results):
    cat0 = np.empty((B, C, H, W), np.float32)
    cat1 = np.empty((B, C, H, W), np.float32)
    for b in range(B):
        o = np.asarray(results[b]["out"]).astype(np.float32).reshape(2, C, H, W)
        cat0[b] = o[0]
        cat1[b] = o[1]
    return (cat0, cat1)


def kernel(**inputs):
    from concourse.bass_utils import run_bass_kernel_spmd

    if "nc" not in _CACHE:
        _CACHE["nc"] = build_nc()
    nc = _CACHE["nc"]
    maps = prep_inputs(inputs)
    res = run_bass_kernel_spmd(nc, maps, core_ids=list(range(B)))
    return postprocess(res.results)



# revision 57
# speedup vs baseline: 1.0126x; 1.0126x over previous
"""Criss-cross attention (2-stream) Trainium2 kernel.

Data-parallel over batch B=8 across 8 NeuronCores; one image pair per core.

Per-core algorithm (all matmuls bf16, fp32 PSUM accumulation):
  - q/k projections for both streams in one pass (bias host-corrected)
  - transposed logits E^T per column (diag mask PRELOADED into PSUM via a
    matmul so the logit matmuls accumulate on top of it) / per row,
    joint softmax without max-subtraction (logits are O(30); exp safe in f32)
  - Z-trick: Z[c',p] = sum_g x[c',g] * Phat[p,g] using host-supplied
    spatially-transposed x copies, then one dense (gamma*wv) @ Z projection.
    v-bias folds out exactly because joint softmax weights sum to 1:
      attn = wv@Z + bv;  out = gamma*attn + x = (gamma*wv)@Z + (x + gamma*bv)
    with x~ = x + gamma*bv supplied by host (and bq' = bq - wq@(gamma*bv),
    bk' = bk - wk@(gamma*bv) correcting the q/k projections).

Perf notes (550us -> ~360us on HW):
  - Cross-attend pipelining: attend 1's col/row logit blocks are emitted
    interleaved with attend 0's zcol/zrow slabs so their exps chase the
    Z reads chunk-by-chunk; proj groups interleave with zrow slabs; the
    mc=1 residual rides PE (identity preload) + ACT copy; stats run on
    DVE reciprocal_approx_fast (~18 correct bits).  Engine queues stay
    fed instead of serializing phase-by-phase.
  - Strided-AP elimination (the dominant unmodeled HW cost): zcol/row
    PSUM outputs are written spatially INTERLEAVED by the PE (strided
    PSUM out APs are ~free) so the ACT/DVE egress reads dense PSUM and
    writes >=4-element runs instead of 2-byte scatters.  This alone was
    worth ~25%.
  - xtc/xtr x-copies ship as fp8 e3m4 (half the DMA bytes; x~N(0,1)
    fits e3m4's range with ~1.5% element error that averages out).
  - exp/ln/identity share one activation-table set (_patch_act_tables).
  - diag mask applied by PE (PSUM preload); FWL-padded row stationaries.
  - eps pool runs 4x single-bank PSUM slots for deeper producer/consumer
    handoff pipelining.
  - attend 0's q/k extract + row blocks chunked into the (DMA-bound)
    projection loop.
  - test.py times via median of same-round pairwise For_i-loop slopes
    (robust to the tunnel's bimodal per-call overhead).
"""

import sys

sys.path.insert(0, "/opt/trn_rl_repo")

import numpy as np
import ml_dtypes


def _patch_act_tables():
    """Make Exp/Ln resolve to the combined natural_log_exp_and_others
    activation-table set so alternating ln/exp does not reload tables.

    bass picks the first set whose membership contains the function; by
    default Exp -> exp_and_others and Ln -> natural_log, which forces a
    ~2.7us table reload on every ln<->exp switch.  Dropping Exp/Ln from
    the other sets' advertised membership makes both resolve to the one
    real hardware set that contains both functions."""
    import functools
    import concourse.hw_specs as hw_specs
    from concourse import mybir

    if getattr(hw_specs.get_activation_tables, "_cc_patched", False):
        return
    orig = hw_specs.get_activation_tables.__wrapped__
    EXP = mybir.ActivationFunctionType.Exp
    LN = mybir.ActivationFunctionType.Ln
    IDENT = mybir.ActivationFunctionType.Identity

    def patched(module_arch):
        out = {}
        for name, fns in orig(module_arch).items():
            fns = set(fns)
            if name != "natural_log_exp_and_others":
                fns.discard(EXP)
                fns.discard(LN)
                fns.discard(IDENT)
            out[name] = fns
        return out

    wrapper = functools.cache(patched)
    wrapper._cc_patched = True
    hw_specs.get_activation_tables = wrapper


_patch_act_tables()

BF = ml_dtypes.bfloat16
F8 = ml_dtypes.float8_e3m4
B, C, H, W = 8, 256, 96, 96
CQ = 32
S = H * W  # 9216
NEG = -1.0e30
SLW = 24  # spatial slices per xT slab

_CACHE = {}
import os as _os
_qe = _os.environ.get("QKFOLD", "0")
DENSE_ZROW = _os.environ.get("DENSE_ZROW", "0") == "1"  # timing ablation only
DENSE_COL = _os.environ.get("DENSE_COL", "0") == "1"    # timing ablation only
QKFOLD = _qe != "0"          # extract4 + qs4/ks4 maintained
QKFOLD_MM = _qe == "2"       # col matmuls consume the folded layout



def build_nc(reps=1, stop_after=99):
    import concourse.tile as tile
    from concourse import bacc, mybir

    f32 = mybir.dt.float32
    bf16 = mybir.dt.bfloat16

    nc = bacc.Bacc("TRN2", target_bir_lowering=False, debug=False, num_devices=8)

    din = {}

    def dparam(name, shape, dt=bf16):
        din[name] = nc.dram_tensor(name, shape, dt, kind="ExternalInput").ap()

    dparam("xa0", [C, S])          # bf16(x0 + g*bv0), channel-major
    dparam("xa1", [C, S])
    f8 = mybir.dt.float8e3
    dparam("xtc0", [H, W * C], f8)  # xtc[h, w*256+c] = x0[c,h,w]  (raw x)
    dparam("xtr0", [W, H * C], f8)  # xtr[w, h*256+c] = x0[c,h,w]
    dparam("xtc1", [H, W * C], f8)
    dparam("xtr1", [W, H * C], f8)
    dparam("wqk", [C, 128])        # cols: wq0T|wk0T|wq1T|wk1T
    dparam("wva", [C, 512])        # (gamma*wv0).T | (gamma*wv1).T
    dparam("qkb", [128, 1], f32)   # bq0'|bk0'|bq1'|bk1'
    dparam("maskw", [H, 128])      # -1e30 diag (cols 96:128 zero)
    dparam("idm", [H, 384])        # identity tiled 4x along cols
    dparam("ident", [128, 128])    # identity (psum residual preload)
    out = nc.dram_tensor("out", [2, C, S], bf16, kind="ExternalOutput").ap()

    with tile.TileContext(nc) as tc:
        if reps == 1:
            _emit(tc, nc, din, out, mybir, stop_after)
        else:
            # the body is far over 256 instructions per engine (≈12 IRAM
            # blocks on PE), so the back-edge target always I$-misses
            # (~3-4us/iter); branch-prefetch hints cut that to ~300ns.
            from concourse.engine_type import EngineType
            with tc.For_i(0, reps, 1,
                          hint_engines=(EngineType.PE, EngineType.Activation,
                                        EngineType.DVE)):
                _emit(tc, nc, din, out, mybir, stop_after)

    nc.compile()
    return nc


def _emit(tc, nc, din, out, mybir, stop_after=99):
    from contextlib import ExitStack

    f32 = mybir.dt.float32
    bf16 = mybir.dt.bfloat16
    f8 = mybir.dt.float8e3
    EXP = mybir.ActivationFunctionType.Exp
    LN = mybir.ActivationFunctionType.Ln
    CPY = mybir.ActivationFunctionType.Copy
    IDENT = mybir.ActivationFunctionType.Identity
    ADD = mybir.AluOpType.add
    MUL = mybir.AluOpType.mult

    SP = S + 128  # padded q/k width (row-logit FWL reads h*96..h*96+128)

    ctx = ExitStack()
    with ctx:
        const = ctx.enter_context(tc.tile_pool(name="const", bufs=1))
        persist = ctx.enter_context(tc.tile_pool(name="persist", bufs=1))
        # PSUM: eps 4x1bank + lps 2x2banks = all 8 banks.  eps runs 4
        # single-bank slots so producer->consumer handoffs pipeline deeper
        # (cross-engine handoff latency is the dominant unmodeled HW cost).
        eps = ctx.enter_context(tc.tile_pool(name="eps", bufs=4, space="PSUM"))
        lps = ctx.enter_context(tc.tile_pool(name="lps", bufs=2, space="PSUM"))

        # ---------------- constants (ACT-triggered HWDGE queue) ----------
        wqk_t = []
        for kc in range(2):
            t = const.tile([128, 128], bf16, tag=f"wqk{kc}", name=f"wqk{kc}")
            nc.scalar.dma_start(t[:], din["wqk"][kc * 128:(kc + 1) * 128, :])
            wqk_t.append(t)
        # wv weights for both streams, one [128, 512] tile per kc half
        wva_t = []
        for kc in range(2):
            t = const.tile([128, 512], bf16, tag=f"wva{kc}", name=f"wva{kc}")
            nc.scalar.dma_start(t[:], din["wva"][kc * 128:(kc + 1) * 128, :])
            wva_t.append(t)
        wv_t = [
            [
                [wva_t[kc][:, s * 256 + mc * 128:s * 256 + (mc + 1) * 128]
                 for mc in range(2)]
                for kc in range(2)
            ]
            for s in range(2)
        ]
        qkb_t = const.tile([128, 1], f32, tag="qkb")
        nc.scalar.dma_start(qkb_t[:], din["qkb"][:])
        maskw_t = const.tile([H, 128], bf16, tag="maskw")
        nc.scalar.dma_start(maskw_t[:], din["maskw"][:])
        idm_t = const.tile([H, 384], bf16, tag="idm")
        nc.scalar.dma_start(idm_t[:], din["idm"][:])
        ones_t = const.tile([H, 128], bf16, tag="ones")
        nc.vector.memset(ones_t[:], 1.0)
        ident_t = const.tile([128, 128], bf16, tag="ident")
        nc.scalar.dma_start(ident_t[:], din["ident"][:])

        # persistent state
        qk_t = persist.tile([128, S], bf16, tag="qk")
        qs = persist.tile([CQ, SP], bf16, tag="qs")
        ks = persist.tile([CQ, SP], bf16, tag="ks")
        ks4 = persist.tile([CQ, S], bf16, tag="ks4") if QKFOLD else None
        pcol = persist.tile([H, S], bf16, tag="pcol")
        prow = persist.tile([W, S], bf16, tag="prow")
        z = [persist.tile([128, S], bf16, tag=f"z{kc}", name=f"z{kc}")
             for kc in range(2)]

        # zero the FWL pad columns once up-front (extract DMAs never touch
        # them)
        nc.vector.memset(qs[:, S:SP], 0.0)
        nc.vector.memset(ks[:, S:SP], 0.0)

        # ---------------- q/k projections (xa streamed in 3K slabs) -----
        # qk_t rows: q0(0:32) k0(32:64) q1(64:96) k1(96:128)
        # attend 0's q/k extract is chunked into this loop, and attend 0's
        # row-logit blocks whose FWL-padded reads fit inside the extracted
        # prefix are emitted right behind each chunk, so PE/ACT start attend
        # 0 while xa is still streaming in.  (Called at emission time below;
        # row_block is late-bound.)
        ROWB = {0: tuple(range(0, 28, 4)), 1: tuple(range(28, 60, 4)),
                2: tuple(range(60, 96, 4))}

        def qk_proj():
            with tc.tile_pool(name="xslp", bufs=2) as xslp:
                for n3 in range(3):
                    xsl = [[None] * 2 for _ in range(2)]
                    for s in range(2):
                        for kc in range(2):
                            t = xslp.tile([128, 3072], bf16, tag=f"x{s}s{kc}",
                                          name=f"x{s}s{kc}")
                            xsl[s][kc] = t
                            nc.sync.dma_start(
                                t[:],
                                din[f"xa{s}"][kc * 128:(kc + 1) * 128,
                                              n3 * 3072:(n3 + 1) * 3072],
                            )
                    for j2 in range(6):
                        jsl = slice(j2 * 512, (j2 + 1) * 512)
                        p = eps.tile([128, 512], f32, tag="eps", name="p")
                        for kc in range(2):
                            nc.tensor.matmul(
                                p[0:64, :], wqk_t[kc][:, 0:64],
                                xsl[0][kc][:, jsl],
                                start=(kc == 0), stop=(kc == 1),
                            )
                        for kc in range(2):
                            nc.tensor.matmul(
                                p[64:128, :], wqk_t[kc][:, 64:128],
                                xsl[1][kc][:, jsl],
                                start=(kc == 0), stop=(kc == 1),
                                tile_position=(0, 64),
                                skip_group_check=True,
                            )
                        # PSUM egress + bias on DVE: ACT's queue holds the
                        # extract-gated row exps, so ACT egress here would
                        # starve the eps slots behind them (head-of-line).
                        nc.vector.tensor_scalar_add(
                            qk_t[:, n3 * 3072 + j2 * 512:
                                 n3 * 3072 + (j2 + 1) * 512], p[:], qkb_t[:]
                        )
                    n3sl = slice(n3 * 3072, (n3 + 1) * 3072)
                    nc.scalar.dma_start(qs[:, n3sl], qk_t[64:96, n3sl])
                    nc.scalar.dma_start(ks[:, n3sl], qk_t[32:64, n3sl])
                    # row blocks ride the (idle-during-qk) lps pool so their
                    # ets — which wait on the extract DMA — never hold the
                    # eps slots the projection's p tiles cycle through.
                    for h0 in ROWB[n3]:
                        row_block(0, h0, lps, "lps")

        # transient pools for the attend phases are entered after qk_proj()
        # runs (xslp's address range frees for them); the closures below
        # late-bind these names.
        slab = tsl = resl = obuf = None

        # ---------------- attends (phase closures, interleaved emission) --
        # pixel (h,w) lives at free index w*96+h in pcol/prow
        prow_hw = prow[:].rearrange("p (w h) -> p h w", h=H)
        prow_wh = prow[:].rearrange("p (w h) -> p w h", h=H)
        qs_wh = qs[:, 0:S].rearrange("p (h w) -> p w h", w=W)
        ks_wh = ks[:, 0:S].rearrange("p (h w) -> p w h", w=W)
        z_wh = [zz[:].rearrange("p (h w) -> p w h", w=W) for zz in z]
        z_hw = [zz[:].rearrange("p (h w) -> p h w", w=W) for zz in z]

        def extract(a):
            # PE requires matmul operands at equal base partitions: copy the
            # attend's q/k blocks to partition-0-based tiles (SBUF->SBUF DMA
            # on the otherwise-idle ACT-triggered queue).
            # (attend 0's extract is chunked into the q/k projection loop.)
            qr = 64 if a == 0 else 0    # query rows (q1 / q0)
            kr = 32 if a == 0 else 96   # key rows (k0 / k1)
            nc.scalar.dma_start(qs[:, 0:S], qk_t[qr:qr + 32, :])
            nc.scalar.dma_start(ks[:, 0:S], qk_t[kr:kr + 32, :])
            if QKFOLD:
                extract4(a)

        def extract4(a):
            # w%4-folded q/k copies for the col branch: free index
            # m*576 + (w//4)*96 + h (m = w%4).  Both col-matmul operands
            # become dense contiguous [32, 96] slices at base partition 0,
            # replacing the stride-96 reads of the w-major views; the
            # extract DMA moves contiguous 96-element runs.
            kr = 32 if a == 0 else 96
            for dst4, r in ((ks4, kr),):
                srcv = qk_t[r:r + 32, :].rearrange(
                    "p (w4 m h) -> p m w4 h", m=4, h=H
                )
                dstv = dst4[:, :].rearrange(
                    "p (m w4 h) -> p m w4 h", m=4, h=H
                )
                for m in range(4):
                    nc.scalar.dma_start(dstv[:, m], srcv[:, m])

        def stats_chunk(a, n2, dve_recip=False):
            sl = slice(n2 * 1024, (n2 + 1) * 1024)
            lt = lps.tile([128, 1024], f32, tag="lps", name="lt")
            for b in range(2):
                s512 = slice(n2 * 1024 + b * 512, n2 * 1024 + (b + 1) * 512)
                bank = lt[:, b * 512:(b + 1) * 512]
                nc.tensor.matmul(
                    bank, ones_t[:], pcol[:, s512],
                    start=True, stop=False, skip_group_check=True,
                )
                nc.tensor.matmul(
                    bank, ones_t[:], prow[:, s512],
                    start=False, stop=True, skip_group_check=True,
                )
            rs = tsl.tile([96, 1024], bf16, tag="rsl")
            if dve_recip:
                rr = tsl.tile([96, 1024], f32, tag="tln", name="rr")
                nc.vector.reciprocal_approx_fast(out=rr[:], in_=lt[0:96, :])
                nc.vector.tensor_copy(rs[:], rr[:])
            else:
                tl = tsl.tile([96, 1024], f32, tag="tln")
                nc.scalar.activation(tl[:], lt[0:96, :], LN)
                nc.scalar.activation(rs[:], tl[:], EXP, scale=-1.0)
            nc.vector.tensor_tensor(pcol[:, sl], pcol[:, sl], rs[:], MUL)
            nc.vector.tensor_tensor(prow[:, sl], prow[:, sl], rs[:], MUL)

        def row_block(a, h0, pool=None, tag="eps"):
            # 4 row-branch logit matmuls (FWL-padded stationary) + exp.
            # PE writes h-interleaved (stride-4 PSUM) so the exp reads dense
            # and writes 4-element runs into w-major prow instead of a
            # 2-byte scatter.
            et = (pool or eps).tile([128, 512], f32, tag=tag, name="et")
            etv = et[:, 0:384].rearrange("p (g j) -> p j g", j=4)
            for j in range(4):
                h = h0 + j
                nc.tensor.matmul(
                    etv[:, j, :],
                    ks[:, h * 96:h * 96 + 128],
                    qs[:, h * 96:(h + 1) * 96],
                    start=True, stop=True, skip_group_check=True,
                )
            src = et[0:96, 0:384].rearrange("p (g j) -> p g j", j=4)
            dst = prow_wh[:, :, h0:h0 + 4]
            nc.scalar.activation(dst, src, EXP)

        def col_block(a, w0, pool, tag):
            # diag-mask PSUM preload + 4 col-branch logit matmuls + exp.
            # With QKFOLD, w0 is a multiple of 4 and the 4 w's live at the 4
            # partition groups of qs4/ks4 with a shared (w//4) free slice,
            # so stationary and moving are dense and at the same base.
            w4 = w0 // 4
            fsl = slice(w4 * 96, (w4 + 1) * 96)
            et = pool.tile([128, 512], f32, tag=tag, name="et")
            nc.tensor.matmul(
                et[:, 0:384], maskw_t[:], idm_t[:],
                start=True, stop=False, skip_group_check=True,
            )
            for j in range(4):
                if QKFOLD_MM:
                    fo = j * 2304 + w4 * 96
                    nc.tensor.matmul(
                        et[0:96, j * 96:j * 96 + 96],
                        ks4[:, fo:fo + 96],
                        qs_wh[:, w0 + j, :],
                        start=False, stop=(j == 3), skip_group_check=True,
                    )
                else:
                    w = w0 + j
                    st = (ks[:, w * 96:w * 96 + 96] if DENSE_COL
                          else ks_wh[:, w, :])
                    mv = (qs[:, w * 96:w * 96 + 96] if DENSE_COL
                          else qs_wh[:, w, :])
                    nc.tensor.matmul(
                        et[0:96, j * 96:j * 96 + 96],
                        st, mv,
                        start=False, stop=(j == 3), skip_group_check=True,
                    )
            src = et[0:96, 0:384]
            nc.scalar.activation(
                pcol[:, w0 * 96:(w0 + 4) * 96], src, EXP
            )

        def zcol_slabs(a):
            # Z column branch: per w, Z[c', h]; scatter w-strided into z.
            # Generator yielding after each w-slab so attend 1's col logit
            # blocks can interleave (they chase this branch's pcol reads).
            xtc = din[f"xtc{a}"][:].rearrange("p (w c) -> p w c", c=C)
            for w0 in range(0, W, SLW):
                xs = slab.tile([H, SLW * 256], f8, tag="xslab", name="xs")
                nc.gpsimd.dma_start(xs[:], xtc[:, w0:w0 + SLW, :])
                for kc in range(2):
                    for g0 in range(0, SLW, 4):
                        zp = eps.tile([128, 512], f32, tag="eps", name="zp")
                        # PE writes w-interleaved (stride-4 PSUM) so the ACT
                        # egress reads dense and writes 4-element runs into
                        # z's h-major layout instead of a 2-byte scatter.
                        zpv = zp[:, 0:384].rearrange("p (g j) -> p j g", j=4)
                        for j in range(4):
                            wl = g0 + j
                            nc.tensor.matmul(
                                zpv[:, j, :],
                                xs[:, wl * 256 + kc * 128:wl * 256 + kc * 128 + 128],
                                pcol[:, (w0 + wl) * 96:(w0 + wl + 1) * 96],
                                start=True, stop=True, skip_group_check=True,
                            )
                        src = zp[:, 0:384].rearrange("p (g j) -> p g j", j=4)
                        dst = z_hw[kc][:, :, w0 + g0:w0 + g0 + 4]
                        nc.scalar.activation(dst, src, CPY)
                yield

        def zrow_slabs(a):
            # Z row branch: per h, Z[c', w]; accumulate into z.  Generator
            # yielding after each h-slab so proj emission can interleave.
            xtr = din[f"xtr{a}"][:].rearrange("p (h c) -> p h c", c=C)
            for h0 in range(0, H, SLW):
                xs = slab.tile([W, SLW * 256], f8, tag="xslab", name="xs")
                nc.sync.dma_start(xs[:], xtr[:, h0:h0 + SLW, :])
                for kc in range(2):
                    for g0 in range(0, SLW, 4):
                        zp = eps.tile([128, 512], f32, tag="eps", name="zp")
                        for j in range(4):
                            hl = g0 + j
                            mv = (prow[:, (h0 + hl) * 96:(h0 + hl + 1) * 96]
                                  if DENSE_ZROW else prow_hw[:, h0 + hl, :])
                            nc.tensor.matmul(
                                zp[:, j * 96:j * 96 + 96],
                                xs[:, hl * 256 + kc * 128:hl * 256 + kc * 128 + 128],
                                mv,
                                start=True, stop=True, skip_group_check=True,
                            )
                        zsl = z[kc][:, (h0 + g0) * 96:(h0 + g0 + 4) * 96]
                        src = zp[:, 0:384]
                        nc.vector.tensor_tensor(zsl, src, zsl, ADD)
                yield

        def proj_groups(a, act_split=False):
            # final projection + residual + store, as a generator yielding
            # after each (n3, mc) output group so callers can interleave other
            # phases' emissions between groups.  n3 is the outer loop so the
            # zrow interleave can release spatially-complete z ranges early.
            for n3 in range(3):
                for mc in range(2):
                    act_path = act_split and mc == 1
                    ob = obuf.tile([128, 3072], bf16, tag="ob")
                    rt = resl.tile([128, 3072], bf16, tag="res", name="rt")
                    nc.sync.dma_start(
                        rt[:],
                        din[f"xa{a}"][mc * 128:(mc + 1) * 128,
                                      n3 * 3072:(n3 + 1) * 3072],
                    )
                    for j2 in range(3):
                        n2 = n3 * 3 + j2
                        op = lps.tile([128, 1024], f32, tag="lps", name="op")
                        for b in range(2):
                            s512 = slice(n2 * 1024 + b * 512,
                                         n2 * 1024 + (b + 1) * 512)
                            opb = op[:, b * 512:(b + 1) * 512]
                            if act_path:
                                nc.tensor.matmul(
                                    opb, ident_t[:],
                                    rt[:, j2 * 1024 + b * 512:
                                       j2 * 1024 + (b + 1) * 512],
                                    start=True, stop=False,
                                    skip_group_check=True,
                                )
                            for kc in range(2):
                                nc.tensor.matmul(
                                    opb, wv_t[a][kc][mc], z[kc][:, s512],
                                    start=(not act_path and kc == 0),
                                    stop=(kc == 1),
                                )
                        jsl = slice(j2 * 1024, (j2 + 1) * 1024)
                        if act_path:
                            nc.scalar.activation(ob[:, jsl], op[:], CPY)
                        else:
                            nc.vector.tensor_tensor(
                                ob[:, jsl], op[:], rt[:, jsl], ADD
                            )
                    nc.gpsimd.dma_start(
                        out[a, mc * 128:(mc + 1) * 128,
                            n3 * 3072:(n3 + 1) * 3072],
                        ob[:],
                    )
                    yield

        def zrow_proj(a, next_rows=False):
            # zrow slabs interleaved with (optionally) the next attend's row
            # logit blocks — whose prow writes chase this branch's per-slab
            # reads — and with proj groups: proj group (n3, mc) needs z rows
            # h < (n3+1)*32 finalized, i.e. zrow slabs through
            # h0 = ceil((n3+1)*32 / SLW).  The residual path for mc=1 rides
            # PE (identity preload) + ACT copy so the DVE queue only carries
            # the zrow adds + mc=0 residuals.
            pg = proj_groups(a, act_split=True)
            zr = zrow_slabs(a)
            for si in range(4):          # slabs h0 = 0, 24, 48, 72
                next(zr)
                if next_rows:
                    for h0 in range(24 * si, 24 * si + 24, 4):
                        row_block(1 - a, h0)
                if si >= 1:
                    next(pg)             # (n3 = si-1, mc = 0)
                    next(pg)             # (n3 = si-1, mc = 1)
            for _ in pg:
                pass

        # Emission program.  The two attends are pipelined across engines:
        #  - attend 0's q/k extract and row-logit blocks are chunked into the
        #    projection loop above, so ACT starts exp'ing while xa streams in.
        #  - attend 0 col blocks + fused stats (recip on DVE, normalize on
        #    GpSimd) follow; then extract(1) (WAR on qs/ks clears when
        #    attend 0's logit matmuls retire).
        #  - zcol(0) slabs interleave with attend 1's col blocks: the col
        #    exps overwrite pcol w-block by w-block right behind zcol(0)'s
        #    w-ordered reads, and attend 1's logit matmuls fill PE while
        #    zcol(0)'s groups pace on ACT egress.  Attend 1's ets live in
        #    the lps pool so the two pipelines don't share PSUM slots.
        #  - zrow(0) slabs interleave with attend 1's row blocks (same
        #    chasing argument) and with proj(0) groups.
        #  - proj residuals for mc=1 ride PE (identity preload) + ACT copy;
        #    mc=0 stays on DVE.
        qk_proj()
        if stop_after <= 1:
            return
        slab = ctx.enter_context(tc.tile_pool(name="slab", bufs=5))
        tsl = ctx.enter_context(tc.tile_pool(name="tsl", bufs=3))
        resl = ctx.enter_context(tc.tile_pool(name="resl", bufs=2))
        obuf = ctx.enter_context(tc.tile_pool(name="obuf", bufs=2))
        # stats0 uses the ACT ln/exp path (its window is DVE-bound with ACT
        # slack); stats1 uses the DVE reciprocal (its window is the reverse).
        n2_done = 0
        for i, w0 in enumerate(range(0, W, 4)):
            col_block(0, w0, eps, "eps")
            cov = (w0 + 4) * 96
            while n2_done < 9 and (n2_done + 1) * 1024 <= cov:
                stats_chunk(0, n2_done, dve_recip=True)
                n2_done += 1
        while n2_done < 9:
            stats_chunk(0, n2_done, dve_recip=True)
            n2_done += 1
        if stop_after <= 2:
            return
        extract(1)
        zc = zcol_slabs(0)
        for si in range(4):
            next(zc)
            for w0 in range(24 * si, 24 * si + 24, 4):
                col_block(1, w0, lps, "lps")
        if stop_after <= 3:
            return
        zrow_proj(0, next_rows=True)
        if stop_after <= 4:
            return
        for n2 in range(9):
            stats_chunk(1, n2, dve_recip=True)
        zc = zcol_slabs(1)
        for si in range(4):
            next(zc)
        if stop_after <= 5:
            return
        zrow_proj(1)


def prep_inputs(inputs):
    """Host-side per-core input prep (numpy)."""
    g = float(np.asarray(inputs["gamma"]).reshape(-1)[0])
    maskw = np.zeros((H, 128), np.float32)
    np.fill_diagonal(maskw[:, 0:H], NEG)
    idm = np.tile(np.eye(H, dtype=np.float32), (1, 4))
    ident = np.eye(128, dtype=np.float32)
    wqk = np.concatenate(
        [inputs["wq0"].T, inputs["wk0"].T, inputs["wq1"].T, inputs["wk1"].T],
        axis=1,
    ).astype(BF)
    wv0 = (g * np.asarray(inputs["wv0"], np.float64)).T.astype(BF)
    wv1 = (g * np.asarray(inputs["wv1"], np.float64)).T.astype(BF)
    gb0 = g * np.asarray(inputs["bv0"], np.float64)
    gb1 = g * np.asarray(inputs["bv1"], np.float64)
    qkb = np.concatenate(
        [
            inputs["bq0"] - inputs["wq0"].astype(np.float64) @ gb0,
            inputs["bk0"] - inputs["wk0"].astype(np.float64) @ gb0,
            inputs["bq1"] - inputs["wq1"].astype(np.float64) @ gb1,
            inputs["bk1"] - inputs["wk1"].astype(np.float64) @ gb1,
        ]
    ).astype(np.float32)[:, None]
    maps = []
    for b in range(B):
        x0 = np.asarray(inputs["x0"][b], np.float32)
        x1 = np.asarray(inputs["x1"][b], np.float32)
        maps.append({
            "xa0": (x0 + np.float32(gb0[:, None, None])).reshape(C, S).astype(BF),
            "xa1": (x1 + np.float32(gb1[:, None, None])).reshape(C, S).astype(BF),
            "xtc0": np.ascontiguousarray(x0.transpose(1, 2, 0)).reshape(H, W * C).astype(F8),
            "xtr0": np.ascontiguousarray(x0.transpose(2, 1, 0)).reshape(W, H * C).astype(F8),
            "xtc1": np.ascontiguousarray(x1.transpose(1, 2, 0)).reshape(H, W * C).astype(F8),
            "xtr1": np.ascontiguousarray(x1.transpose(2, 1, 0)).reshape(W, H * C).astype(F8),
            "wqk": wqk, "wva": np.concatenate([wv0, wv1], axis=1), "qkb": qkb,
            "maskw": maskw.astype(BF), "idm": idm.astype(BF),
            "ident": ident.astype(BF),
        })
    return maps


def postprocess(results):
    cat0 = np.empty((B, C, H, W), np.float32)
    cat1 = np.empty((B, C, H, W), np.float32)
    for b in range(B):
        o = np.asarray(results[b]["out"]).astype(np.float32).reshape(2, C, H, W)
        cat0[b] = o[0]
        cat1[b] = o[1]
    return (cat0, cat1)


def kernel(**inputs):
    from concourse.bass_utils import run_bass_kernel_spmd

    if "nc" not in _CACHE:
        _CACHE["nc"] = build_nc()
    nc = _CACHE["nc"]
    maps = prep_inputs(inputs)
    res = run_bass_kernel_spmd(nc, maps, core_ids=list(range(B)))
    return postprocess(res.results)



# revision 58
# speedup vs baseline: 17.3046x; 17.0896x over previous
"""Criss-cross attention (2-stream) Trainium2 kernel.

Data-parallel over batch B=8 across 8 NeuronCores; one image pair per core.

Per-core algorithm (all matmuls bf16, fp32 PSUM accumulation):
  - q/k projections for both streams in one pass (bias host-corrected)
  - transposed logits E^T per column (diag mask PRELOADED into PSUM via a
    matmul so the logit matmuls accumulate on top of it) / per row,
    joint softmax without max-subtraction (logits are O(30); exp safe in f32)
  - Z-trick: Z[c',p] = sum_g x[c',g] * Phat[p,g] using host-supplied
    spatially-transposed x copies, then one dense (gamma*wv) @ Z projection.
    v-bias folds out exactly because joint softmax weights sum to 1:
      attn = wv@Z + bv;  out = gamma*attn + x = (gamma*wv)@Z + (x + gamma*bv)
    with x~ = x + gamma*bv supplied by host (and bq' = bq - wq@(gamma*bv),
    bk' = bk - wk@(gamma*bv) correcting the q/k projections).

Perf notes (550us -> ~360us on HW):
  - Cross-attend pipelining: attend 1's col/row logit blocks are emitted
    interleaved with attend 0's zcol/zrow slabs so their exps chase the
    Z reads chunk-by-chunk; proj groups interleave with zrow slabs; the
    mc=1 residual rides PE (identity preload) + ACT copy; stats run on
    DVE reciprocal_approx_fast (~18 correct bits).  Engine queues stay
    fed instead of serializing phase-by-phase.
  - Strided-AP elimination (the dominant unmodeled HW cost): zcol/row
    PSUM outputs are written spatially INTERLEAVED by the PE (strided
    PSUM out APs are ~free) so the ACT/DVE egress reads dense PSUM and
    writes >=4-element runs instead of 2-byte scatters.  This alone was
    worth ~25%.
  - xtc/xtr x-copies ship as fp8 e3m4 (half the DMA bytes; x~N(0,1)
    fits e3m4's range with ~1.5% element error that averages out).
  - exp/ln/identity share one activation-table set (_patch_act_tables).
  - diag mask applied by PE (PSUM preload); FWL-padded row stationaries.
  - eps pool runs 4x single-bank PSUM slots for deeper producer/consumer
    handoff pipelining.
  - attend 0's q/k extract + row blocks chunked into the (DMA-bound)
    projection loop.
  - test.py times via median of same-round pairwise For_i-loop slopes
    (robust to the tunnel's bimodal per-call overhead).
"""

import sys

sys.path.insert(0, "/opt/trn_rl_repo")

import numpy as np
import ml_dtypes


def _patch_act_tables():
    """Make Exp/Ln resolve to the combined natural_log_exp_and_others
    activation-table set so alternating ln/exp does not reload tables.

    bass picks the first set whose membership contains the function; by
    default Exp -> exp_and_others and Ln -> natural_log, which forces a
    ~2.7us table reload on every ln<->exp switch.  Dropping Exp/Ln from
    the other sets' advertised membership makes both resolve to the one
    real hardware set that contains both functions."""
    import functools
    import concourse.hw_specs as hw_specs
    from concourse import mybir

    if getattr(hw_specs.get_activation_tables, "_cc_patched", False):
        return
    orig = hw_specs.get_activation_tables.__wrapped__
    EXP = mybir.ActivationFunctionType.Exp
    LN = mybir.ActivationFunctionType.Ln
    IDENT = mybir.ActivationFunctionType.Identity

    def patched(module_arch):
        out = {}
        for name, fns in orig(module_arch).items():
            fns = set(fns)
            if name != "natural_log_exp_and_others":
                fns.discard(EXP)
                fns.discard(LN)
                fns.discard(IDENT)
            out[name] = fns
        return out

    wrapper = functools.cache(patched)
    wrapper._cc_patched = True
    hw_specs.get_activation_tables = wrapper


_patch_act_tables()

BF = ml_dtypes.bfloat16
F8 = ml_dtypes.float8_e3m4
B, C, H, W = 8, 256, 96, 96
CQ = 32
S = H * W  # 9216
NEG = -1.0e30
SLW = 24  # spatial slices per xT slab

_CACHE = {}
import os as _os
_qe = _os.environ.get("QKFOLD", "0")
DENSE_ZROW = _os.environ.get("DENSE_ZROW", "0") == "1"  # timing ablation only
DENSE_COL = _os.environ.get("DENSE_COL", "0") == "1"    # timing ablation only
QKFOLD = _qe != "0"          # extract4 + qs4/ks4 maintained
QKFOLD_MM = _qe == "2"       # col matmuls consume the folded layout



def build_nc(reps=1, stop_after=99):
    import concourse.tile as tile
    from concourse import bacc, mybir

    f32 = mybir.dt.float32
    bf16 = mybir.dt.bfloat16

    nc = bacc.Bacc("TRN2", target_bir_lowering=False, debug=False, num_devices=8)

    din = {}

    def dparam(name, shape, dt=bf16):
        din[name] = nc.dram_tensor(name, shape, dt, kind="ExternalInput").ap()

    dparam("xa0", [C, S])          # bf16(x0 + g*bv0), channel-major
    dparam("xa1", [C, S])
    f8 = mybir.dt.float8e3
    dparam("xtc0", [H, W * C], f8)  # xtc[h, w*256+c] = x0[c,h,w]  (raw x)
    dparam("xtr0", [W, H * C], f8)  # xtr[w, h*256+c] = x0[c,h,w]
    dparam("xtc1", [H, W * C], f8)
    dparam("xtr1", [W, H * C], f8)
    dparam("wqk", [C, 128])        # cols: wq0T|wk0T|wq1T|wk1T
    dparam("wva", [C, 512])        # (gamma*wv0).T | (gamma*wv1).T
    dparam("qkb", [128, 1], f32)   # bq0'|bk0'|bq1'|bk1'
    dparam("maskw", [H, 128])      # -1e30 diag (cols 96:128 zero)
    dparam("idm", [H, 384])        # identity tiled 4x along cols
    dparam("ident", [128, 128])    # identity (psum residual preload)
    out = nc.dram_tensor("out", [2, C, S], bf16, kind="ExternalOutput").ap()

    with tile.TileContext(nc) as tc:
        if reps == 1:
            _emit(tc, nc, din, out, mybir, stop_after)
        else:
            # the body is far over 256 instructions per engine (≈12 IRAM
            # blocks on PE), so the back-edge target always I$-misses
            # (~3-4us/iter); branch-prefetch hints cut that to ~300ns.
            from concourse.engine_type import EngineType
            with tc.For_i(0, reps, 1,
                          hint_engines=(EngineType.PE, EngineType.Activation,
                                        EngineType.DVE)):
                _emit(tc, nc, din, out, mybir, stop_after)

    nc.compile()
    return nc


def _emit(tc, nc, din, out, mybir, stop_after=99):
    from contextlib import ExitStack

    f32 = mybir.dt.float32
    bf16 = mybir.dt.bfloat16
    f8 = mybir.dt.float8e3
    EXP = mybir.ActivationFunctionType.Exp
    LN = mybir.ActivationFunctionType.Ln
    CPY = mybir.ActivationFunctionType.Copy
    IDENT = mybir.ActivationFunctionType.Identity
    ADD = mybir.AluOpType.add
    MUL = mybir.AluOpType.mult

    SP = S + 128  # padded q/k width (row-logit FWL reads h*96..h*96+128)

    ctx = ExitStack()
    with ctx:
        const = ctx.enter_context(tc.tile_pool(name="const", bufs=1))
        persist = ctx.enter_context(tc.tile_pool(name="persist", bufs=1))
        # PSUM: eps 4x1bank + lps 2x2banks = all 8 banks.  eps runs 4
        # single-bank slots so producer->consumer handoffs pipeline deeper
        # (cross-engine handoff latency is the dominant unmodeled HW cost).
        eps = ctx.enter_context(tc.tile_pool(name="eps", bufs=4, space="PSUM"))
        lps = ctx.enter_context(tc.tile_pool(name="lps", bufs=2, space="PSUM"))

        # ---------------- constants (ACT-triggered HWDGE queue) ----------
        wqk_t = []
        for kc in range(2):
            t = const.tile([128, 128], bf16, tag=f"wqk{kc}", name=f"wqk{kc}")
            nc.scalar.dma_start(t[:], din["wqk"][kc * 128:(kc + 1) * 128, :])
            wqk_t.append(t)
        # wv weights for both streams, one [128, 512] tile per kc half
        wva_t = []
        for kc in range(2):
            t = const.tile([128, 512], bf16, tag=f"wva{kc}", name=f"wva{kc}")
            nc.scalar.dma_start(t[:], din["wva"][kc * 128:(kc + 1) * 128, :])
            wva_t.append(t)
        wv_t = [
            [
                [wva_t[kc][:, s * 256 + mc * 128:s * 256 + (mc + 1) * 128]
                 for mc in range(2)]
                for kc in range(2)
            ]
            for s in range(2)
        ]
        qkb_t = const.tile([128, 1], f32, tag="qkb")
        nc.scalar.dma_start(qkb_t[:], din["qkb"][:])
        maskw_t = const.tile([H, 128], bf16, tag="maskw")
        nc.scalar.dma_start(maskw_t[:], din["maskw"][:])
        idm_t = const.tile([H, 384], bf16, tag="idm")
        nc.scalar.dma_start(idm_t[:], din["idm"][:])
        ones_t = const.tile([H, 128], bf16, tag="ones")
        nc.vector.memset(ones_t[:], 1.0)
        ident_t = const.tile([128, 128], bf16, tag="ident")
        nc.scalar.dma_start(ident_t[:], din["ident"][:])

        # persistent state
        qk_t = persist.tile([128, S], bf16, tag="qk")
        qs = persist.tile([CQ, SP], bf16, tag="qs")
        ks = persist.tile([CQ, SP], bf16, tag="ks")
        ks4 = persist.tile([CQ, S], bf16, tag="ks4") if QKFOLD else None
        pcol = persist.tile([H, S], bf16, tag="pcol")
        prow = persist.tile([W, S], bf16, tag="prow")
        z = [persist.tile([128, S], bf16, tag=f"z{kc}", name=f"z{kc}")
             for kc in range(2)]

        # zero the FWL pad columns once up-front (extract DMAs never touch
        # them)
        nc.vector.memset(qs[:, S:SP], 0.0)
        nc.vector.memset(ks[:, S:SP], 0.0)

        # ---------------- q/k projections (xa streamed in 3K slabs) -----
        # qk_t rows: q0(0:32) k0(32:64) q1(64:96) k1(96:128)
        # attend 0's q/k extract is chunked into this loop, and attend 0's
        # row-logit blocks whose FWL-padded reads fit inside the extracted
        # prefix are emitted right behind each chunk, so PE/ACT start attend
        # 0 while xa is still streaming in.  (Called at emission time below;
        # row_block is late-bound.)
        ROWB = {0: tuple(range(0, 28, 4)), 1: tuple(range(28, 60, 4)),
                2: tuple(range(60, 96, 4))}

        def qk_proj():
            with tc.tile_pool(name="xslp", bufs=2) as xslp:
                for n3 in range(3):
                    xsl = [[None] * 2 for _ in range(2)]
                    for s in range(2):
                        for kc in range(2):
                            t = xslp.tile([128, 3072], bf16, tag=f"x{s}s{kc}",
                                          name=f"x{s}s{kc}")
                            xsl[s][kc] = t
                            nc.sync.dma_start(
                                t[:],
                                din[f"xa{s}"][kc * 128:(kc + 1) * 128,
                                              n3 * 3072:(n3 + 1) * 3072],
                            )
                    for j2 in range(6):
                        jsl = slice(j2 * 512, (j2 + 1) * 512)
                        p = eps.tile([128, 512], f32, tag="eps", name="p")
                        for kc in range(2):
                            nc.tensor.matmul(
                                p[0:64, :], wqk_t[kc][:, 0:64],
                                xsl[0][kc][:, jsl],
                                start=(kc == 0), stop=(kc == 1),
                            )
                        for kc in range(2):
                            nc.tensor.matmul(
                                p[64:128, :], wqk_t[kc][:, 64:128],
                                xsl[1][kc][:, jsl],
                                start=(kc == 0), stop=(kc == 1),
                                tile_position=(0, 64),
                                skip_group_check=True,
                            )
                        # PSUM egress + bias on DVE: ACT's queue holds the
                        # extract-gated row exps, so ACT egress here would
                        # starve the eps slots behind them (head-of-line).
                        nc.vector.tensor_scalar_add(
                            qk_t[:, n3 * 3072 + j2 * 512:
                                 n3 * 3072 + (j2 + 1) * 512], p[:], qkb_t[:]
                        )
                    n3sl = slice(n3 * 3072, (n3 + 1) * 3072)
                    nc.scalar.dma_start(qs[:, n3sl], qk_t[64:96, n3sl])
                    nc.scalar.dma_start(ks[:, n3sl], qk_t[32:64, n3sl])
                    # row blocks ride the (idle-during-qk) lps pool so their
                    # ets — which wait on the extract DMA — never hold the
                    # eps slots the projection's p tiles cycle through.
                    for h0 in ROWB[n3]:
                        row_block(0, h0, lps, "lps")

        # transient pools for the attend phases are entered after qk_proj()
        # runs (xslp's address range frees for them); the closures below
        # late-bind these names.
        slab = tsl = resl = obuf = None

        # ---------------- attends (phase closures, interleaved emission) --
        # pixel (h,w) lives at free index w*96+h in pcol/prow
        prow_hw = prow[:].rearrange("p (w h) -> p h w", h=H)
        prow_wh = prow[:].rearrange("p (w h) -> p w h", h=H)
        qs_wh = qs[:, 0:S].rearrange("p (h w) -> p w h", w=W)
        ks_wh = ks[:, 0:S].rearrange("p (h w) -> p w h", w=W)
        z_wh = [zz[:].rearrange("p (h w) -> p w h", w=W) for zz in z]
        z_hw = [zz[:].rearrange("p (h w) -> p h w", w=W) for zz in z]

        def extract(a):
            # PE requires matmul operands at equal base partitions: copy the
            # attend's q/k blocks to partition-0-based tiles (SBUF->SBUF DMA
            # on the otherwise-idle ACT-triggered queue).
            # (attend 0's extract is chunked into the q/k projection loop.)
            qr = 64 if a == 0 else 0    # query rows (q1 / q0)
            kr = 32 if a == 0 else 96   # key rows (k0 / k1)
            nc.scalar.dma_start(qs[:, 0:S], qk_t[qr:qr + 32, :])
            nc.scalar.dma_start(ks[:, 0:S], qk_t[kr:kr + 32, :])
            if QKFOLD:
                extract4(a)

        def extract4(a):
            # w%4-folded q/k copies for the col branch: free index
            # m*576 + (w//4)*96 + h (m = w%4).  Both col-matmul operands
            # become dense contiguous [32, 96] slices at base partition 0,
            # replacing the stride-96 reads of the w-major views; the
            # extract DMA moves contiguous 96-element runs.
            kr = 32 if a == 0 else 96
            for dst4, r in ((ks4, kr),):
                srcv = qk_t[r:r + 32, :].rearrange(
                    "p (w4 m h) -> p m w4 h", m=4, h=H
                )
                dstv = dst4[:, :].rearrange(
                    "p (m w4 h) -> p m w4 h", m=4, h=H
                )
                for m in range(4):
                    nc.scalar.dma_start(dstv[:, m], srcv[:, m])

        def stats_chunk(a, n2, dve_recip=False):
            sl = slice(n2 * 1024, (n2 + 1) * 1024)
            lt = lps.tile([128, 1024], f32, tag="lps", name="lt")
            for b in range(2):
                s512 = slice(n2 * 1024 + b * 512, n2 * 1024 + (b + 1) * 512)
                bank = lt[:, b * 512:(b + 1) * 512]
                nc.tensor.matmul(
                    bank, ones_t[:], pcol[:, s512],
                    start=True, stop=False, skip_group_check=True,
                )
                nc.tensor.matmul(
                    bank, ones_t[:], prow[:, s512],
                    start=False, stop=True, skip_group_check=True,
                )
            rs = tsl.tile([96, 1024], bf16, tag="rsl")
            if dve_recip:
                rr = tsl.tile([96, 1024], f32, tag="tln", name="rr")
                nc.vector.reciprocal_approx_fast(out=rr[:], in_=lt[0:96, :])
                nc.vector.tensor_copy(rs[:], rr[:])
            else:
                tl = tsl.tile([96, 1024], f32, tag="tln")
                nc.scalar.activation(tl[:], lt[0:96, :], LN)
                nc.scalar.activation(rs[:], tl[:], EXP, scale=-1.0)
            nc.vector.tensor_tensor(pcol[:, sl], pcol[:, sl], rs[:], MUL)
            nc.vector.tensor_tensor(prow[:, sl], prow[:, sl], rs[:], MUL)

        def row_block(a, h0, pool=None, tag="eps"):
            # 4 row-branch logit matmuls (FWL-padded stationary) + exp.
            # PE writes h-interleaved (stride-4 PSUM) so the exp reads dense
            # and writes 4-element runs into w-major prow instead of a
            # 2-byte scatter.
            et = (pool or eps).tile([128, 512], f32, tag=tag, name="et")
            etv = et[:, 0:384].rearrange("p (g j) -> p j g", j=4)
            for j in range(4):
                h = h0 + j
                nc.tensor.matmul(
                    etv[:, j, :],
                    ks[:, h * 96:h * 96 + 128],
                    qs[:, h * 96:(h + 1) * 96],
                    start=True, stop=True, skip_group_check=True,
                )
            src = et[0:96, 0:384].rearrange("p (g j) -> p g j", j=4)
            dst = prow_wh[:, :, h0:h0 + 4]
            nc.scalar.activation(dst, src, EXP)

        def col_block(a, w0, pool, tag):
            # diag-mask PSUM preload + 4 col-branch logit matmuls + exp.
            # With QKFOLD, w0 is a multiple of 4 and the 4 w's live at the 4
            # partition groups of qs4/ks4 with a shared (w//4) free slice,
            # so stationary and moving are dense and at the same base.
            w4 = w0 // 4
            fsl = slice(w4 * 96, (w4 + 1) * 96)
            et = pool.tile([128, 512], f32, tag=tag, name="et")
            nc.tensor.matmul(
                et[:, 0:384], maskw_t[:], idm_t[:],
                start=True, stop=False, skip_group_check=True,
            )
            for j in range(4):
                if QKFOLD_MM:
                    fo = j * 2304 + w4 * 96
                    nc.tensor.matmul(
                        et[0:96, j * 96:j * 96 + 96],
                        ks4[:, fo:fo + 96],
                        qs_wh[:, w0 + j, :],
                        start=False, stop=(j == 3), skip_group_check=True,
                    )
                else:
                    w = w0 + j
                    st = (ks[:, w * 96:w * 96 + 96] if DENSE_COL
                          else ks_wh[:, w, :])
                    mv = (qs[:, w * 96:w * 96 + 96] if DENSE_COL
                          else qs_wh[:, w, :])
                    nc.tensor.matmul(
                        et[0:96, j * 96:j * 96 + 96],
                        st, mv,
                        start=False, stop=(j == 3), skip_group_check=True,
                    )
            src = et[0:96, 0:384]
            nc.scalar.activation(
                pcol[:, w0 * 96:(w0 + 4) * 96], src, EXP
            )

        def zcol_slabs(a):
            # Z column branch: per w, Z[c', h]; scatter w-strided into z.
            # Generator yielding after each w-slab so attend 1's col logit
            # blocks can interleave (they chase this branch's pcol reads).
            xtc = din[f"xtc{a}"][:].rearrange("p (w c) -> p w c", c=C)
            for w0 in range(0, W, SLW):
                xs = slab.tile([H, SLW * 256], f8, tag="xslab", name="xs")
                nc.scalar.dma_start(xs[:], xtc[:, w0:w0 + SLW, :])
                for kc in range(2):
                    for g0 in range(0, SLW, 4):
                        zp = eps.tile([128, 512], f32, tag="eps", name="zp")
                        # PE writes w-interleaved (stride-4 PSUM) so the ACT
                        # egress reads dense and writes 4-element runs into
                        # z's h-major layout instead of a 2-byte scatter.
                        zpv = zp[:, 0:384].rearrange("p (g j) -> p j g", j=4)
                        for j in range(4):
                            wl = g0 + j
                            nc.tensor.matmul(
                                zpv[:, j, :],
                                xs[:, wl * 256 + kc * 128:wl * 256 + kc * 128 + 128],
                                pcol[:, (w0 + wl) * 96:(w0 + wl + 1) * 96],
                                start=True, stop=True, skip_group_check=True,
                            )
                        src = zp[:, 0:384].rearrange("p (g j) -> p g j", j=4)
                        dst = z_hw[kc][:, :, w0 + g0:w0 + g0 + 4]
                        nc.scalar.activation(dst, src, CPY)
                yield

        def zrow_slabs(a):
            # Z row branch: per h, Z[c', w]; accumulate into z.  Generator
            # yielding after each h-slab so proj emission can interleave.
            xtr = din[f"xtr{a}"][:].rearrange("p (h c) -> p h c", c=C)
            for h0 in range(0, H, SLW):
                xs = slab.tile([W, SLW * 256], f8, tag="xslab", name="xs")
                nc.sync.dma_start(xs[:], xtr[:, h0:h0 + SLW, :])
                for kc in range(2):
                    for g0 in range(0, SLW, 4):
                        zp = eps.tile([128, 512], f32, tag="eps", name="zp")
                        for j in range(4):
                            hl = g0 + j
                            mv = (prow[:, (h0 + hl) * 96:(h0 + hl + 1) * 96]
                                  if DENSE_ZROW else prow_hw[:, h0 + hl, :])
                            nc.tensor.matmul(
                                zp[:, j * 96:j * 96 + 96],
                                xs[:, hl * 256 + kc * 128:hl * 256 + kc * 128 + 128],
                                mv,
                                start=True, stop=True, skip_group_check=True,
                            )
                        zsl = z[kc][:, (h0 + g0) * 96:(h0 + g0 + 4) * 96]
                        src = zp[:, 0:384]
                        nc.vector.tensor_tensor(zsl, src, zsl, ADD)
                yield

        def proj_groups(a, act_split=False):
            # final projection + residual + store, as a generator yielding
            # after each (n3, mc) output group so callers can interleave other
            # phases' emissions between groups.  n3 is the outer loop so the
            # zrow interleave can release spatially-complete z ranges early.
            for n3 in range(3):
                for mc in range(2):
                    act_path = act_split and mc == 1
                    ob = obuf.tile([128, 3072], bf16, tag="ob")
                    rt = resl.tile([128, 3072], bf16, tag="res", name="rt")
                    nc.sync.dma_start(
                        rt[:],
                        din[f"xa{a}"][mc * 128:(mc + 1) * 128,
                                      n3 * 3072:(n3 + 1) * 3072],
                    )
                    for j2 in range(3):
                        n2 = n3 * 3 + j2
                        op = lps.tile([128, 1024], f32, tag="lps", name="op")
                        for b in range(2):
                            s512 = slice(n2 * 1024 + b * 512,
                                         n2 * 1024 + (b + 1) * 512)
                            opb = op[:, b * 512:(b + 1) * 512]
                            if act_path:
                                nc.tensor.matmul(
                                    opb, ident_t[:],
                                    rt[:, j2 * 1024 + b * 512:
                                       j2 * 1024 + (b + 1) * 512],
                                    start=True, stop=False,
                                    skip_group_check=True,
                                )
                            for kc in range(2):
                                nc.tensor.matmul(
                                    opb, wv_t[a][kc][mc], z[kc][:, s512],
                                    start=(not act_path and kc == 0),
                                    stop=(kc == 1),
                                )
                        jsl = slice(j2 * 1024, (j2 + 1) * 1024)
                        if act_path:
                            nc.scalar.activation(ob[:, jsl], op[:], CPY)
                        else:
                            nc.vector.tensor_tensor(
                                ob[:, jsl], op[:], rt[:, jsl], ADD
                            )
                    nc.sync.dma_start(
                        out[a, mc * 128:(mc + 1) * 128,
                            n3 * 3072:(n3 + 1) * 3072],
                        ob[:],
                    )
                    yield

        def zrow_proj(a, next_rows=False):
            # zrow slabs interleaved with (optionally) the next attend's row
            # logit blocks — whose prow writes chase this branch's per-slab
            # reads — and with proj groups: proj group (n3, mc) needs z rows
            # h < (n3+1)*32 finalized, i.e. zrow slabs through
            # h0 = ceil((n3+1)*32 / SLW).  The residual path for mc=1 rides
            # PE (identity preload) + ACT copy so the DVE queue only carries
            # the zrow adds + mc=0 residuals.
            pg = proj_groups(a, act_split=True)
            zr = zrow_slabs(a)
            for si in range(4):          # slabs h0 = 0, 24, 48, 72
                next(zr)
                if next_rows:
                    for h0 in range(24 * si, 24 * si + 24, 4):
                        row_block(1 - a, h0)
                if si >= 1:
                    next(pg)             # (n3 = si-1, mc = 0)
                    next(pg)             # (n3 = si-1, mc = 1)
            for _ in pg:
                pass

        # Emission program.  The two attends are pipelined across engines:
        #  - attend 0's q/k extract and row-logit blocks are chunked into the
        #    projection loop above, so ACT starts exp'ing while xa streams in.
        #  - attend 0 col blocks + fused stats (recip on DVE, normalize on
        #    GpSimd) follow; then extract(1) (WAR on qs/ks clears when
        #    attend 0's logit matmuls retire).
        #  - zcol(0) slabs interleave with attend 1's col blocks: the col
        #    exps overwrite pcol w-block by w-block right behind zcol(0)'s
        #    w-ordered reads, and attend 1's logit matmuls fill PE while
        #    zcol(0)'s groups pace on ACT egress.  Attend 1's ets live in
        #    the lps pool so the two pipelines don't share PSUM slots.
        #  - zrow(0) slabs interleave with attend 1's row blocks (same
        #    chasing argument) and with proj(0) groups.
        #  - proj residuals for mc=1 ride PE (identity preload) + ACT copy;
        #    mc=0 stays on DVE.
        qk_proj()
        if stop_after <= 1:
            return
        slab = ctx.enter_context(tc.tile_pool(name="slab", bufs=5))
        tsl = ctx.enter_context(tc.tile_pool(name="tsl", bufs=3))
        resl = ctx.enter_context(tc.tile_pool(name="resl", bufs=2))
        obuf = ctx.enter_context(tc.tile_pool(name="obuf", bufs=2))
        # stats0 uses the ACT ln/exp path (its window is DVE-bound with ACT
        # slack); stats1 uses the DVE reciprocal (its window is the reverse).
        n2_done = 0
        for i, w0 in enumerate(range(0, W, 4)):
            col_block(0, w0, eps, "eps")
            cov = (w0 + 4) * 96
            while n2_done < 9 and (n2_done + 1) * 1024 <= cov:
                stats_chunk(0, n2_done, dve_recip=True)
                n2_done += 1
        while n2_done < 9:
            stats_chunk(0, n2_done, dve_recip=True)
            n2_done += 1
        if stop_after <= 2:
            return
        extract(1)
        zc = zcol_slabs(0)
        for si in range(4):
            next(zc)
            for w0 in range(24 * si, 24 * si + 24, 4):
                col_block(1, w0, lps, "lps")
        if stop_after <= 3:
            return
        zrow_proj(0, next_rows=True)
        if stop_after <= 4:
            return
        for n2 in range(9):
            stats_chunk(1, n2, dve_recip=True)
        zc = zcol_slabs(1)
        for si in range(4):
            next(zc)
        if stop_after <= 5:
            return
        zrow_proj(1)


def prep_inputs(inputs):
    """Host-side per-core input prep (numpy)."""
    g = float(np.asarray(inputs["gamma"]).reshape(-1)[0])
    maskw = np.zeros((H, 128), np.float32)
    np.fill_diagonal(maskw[:, 0:H], NEG)
    idm = np.tile(np.eye(H, dtype=np.float32), (1, 4))
    ident = np.eye(128, dtype=np.float32)
    wqk = np.concatenate(
        [inputs["wq0"].T, inputs["wk0"].T, inputs["wq1"].T, inputs["wk1"].T],
        axis=1,
    ).astype(BF)
    wv0 = (g * np.asarray(inputs["wv0"], np.float64)).T.astype(BF)
    wv1 = (g * np.asarray(inputs["wv1"], np.float64)).T.astype(BF)
    gb0 = g * np.asarray(inputs["bv0"], np.float64)
    gb1 = g * np.asarray(inputs["bv1"], np.float64)
    qkb = np.concatenate(
        [
            inputs["bq0"] - inputs["wq0"].astype(np.float64) @ gb0,
            inputs["bk0"] - inputs["wk0"].astype(np.float64) @ gb0,
            inputs["bq1"] - inputs["wq1"].astype(np.float64) @ gb1,
            inputs["bk1"] - inputs["wk1"].astype(np.float64) @ gb1,
        ]
    ).astype(np.float32)[:, None]
    maps = []
    for b in range(B):
        x0 = np.asarray(inputs["x0"][b], np.float32)
        x1 = np.asarray(inputs["x1"][b], np.float32)
        maps.append({
            "xa0": (x0 + np.float32(gb0[:, None, None])).reshape(C, S).astype(BF),
            "xa1": (x1 + np.float32(gb1[:, None, None])).reshape(C, S).astype(BF),
            "xtc0": np.ascontiguousarray(x0.transpose(1, 2, 0)).reshape(H, W * C).astype(F8),
            "xtr0": np.ascontiguousarray(x0.transpose(2, 1, 0)).reshape(W, H * C).astype(F8),
            "xtc1": np.ascontiguousarray(x1.transpose(1, 2, 0)).reshape(H, W * C).astype(F8),
            "xtr1": np.ascontiguousarray(x1.transpose(2, 1, 0)).reshape(W, H * C).astype(F8),
            "wqk": wqk, "wva": np.concatenate([wv0, wv1], axis=1), "qkb": qkb,
            "maskw": maskw.astype(BF), "idm": idm.astype(BF),
            "ident": ident.astype(BF),
        })
    return maps


def postprocess(results):
    cat0 = np.empty((B, C, H, W), np.float32)
    cat1 = np.empty((B, C, H, W), np.float32)
    for b in range(B):
        o = np.asarray(results[b]["out"]).astype(np.float32).reshape(2, C, H, W)
        cat0[b] = o[0]
        cat1[b] = o[1]
    return (cat0, cat1)


def kernel(**inputs):
    from concourse.bass_utils import run_bass_kernel_spmd

    if "nc" not in _CACHE:
        _CACHE["nc"] = build_nc()
    nc = _CACHE["nc"]
    maps = prep_inputs(inputs)
    res = run_bass_kernel_spmd(nc, maps, core_ids=list(range(B)))
    return postprocess(res.results)

